# revision 1
# baseline (speedup 1.0000x reference)
"""Trainium2 Bass kernel for CKANConv2d (KAN conv: SiLU base + B-spline path).

Math: for each output pixel p and output channel co:
  out[co,p] = sum_{c,kh,kw} silu(x[c,p+k]) * Wb[co,(c,kh,kw)]
            + sum_{c,kh,kw,g} B_g(x[c,p+k]) * Ws[co,(c,kh,kw),g]
with B_g the order-3 uniform B-spline bases over knots {-2.2 + 0.4j}.

Key identity used on-chip (t = 2.5*x, center c_g = g - 3.5):
  v = |2.5 x - c_g|; m = min(v,2)-2; n = min(v,1)-1
  B_g(x) = (4 n^3 - m^3) / 6
The 1/6 is folded into the spline weights; the bases are computed per
*input* pixel (not per unfolded patch, 9x less work) and the 3x3
convolution is done as an implicit GEMM over 9 shifted windows with
contraction (c,g) packed 128 rows at a time.

Pipelining: every per-body tile rotates between 2 buffers (tag bufs=2),
so in the replicated timing build body i+1's input DMA + elementwise
basis computation overlap body i's tail matmuls and the PE never
starves. Base matmuls are emitted per-group so each group's PE block
depends only on that group's chunk of elementwise output.

Sharding: data-parallel over batch, 1 image per NeuronCore (8 cores).
"""
import numpy as np
import ml_dtypes

B, CIN, H, W = 8, 64, 56, 56
COUT, K = 128, 3
HO = WO = 54
NOUT = HO * WO  # 2916
NTAP = K * K  # 9
NKT = 4  # spline K-tiles per tap: 128 rows = 64c x 2g, 4 tiles cover g=0..7
NGRP = 6  # output row groups of 9 rows each
GROW = 9  # output rows per group
NFREE = GROW * WO  # 486 <= 512 (one PSUM bank)
RS = 11  # first silu/chunk-0 row boundary
RR = 12  # first x-DMA rows (chunk 0 + 1 shifted-silu row)

_CACHE = {}


def _patch_tile_tail_drain():
    """walrus in this env rejects the Tile tail Drain when it carries >1
    sync waits; split them into a chain of single-wait Drains."""
    import concourse.tile as tile
    from concourse.vector_clock import ScopedClock

    if getattr(tile.TileContext, "_drain_patched", False):
        return

    def _patched(self, tick_clock, wait_clock):
        drain_inst = self.nc.sync.drain()
        wait_clock.add_sem_waits(
            drain_inst.ins, ScopedClock({None: tick_clock.global_clock})
        )
        si = drain_inst.ins.sync_info
        waits = list(si.on_wait) if si is not None else []
        if len(waits) > 1:
            si.on_wait = waits[:1]
            handles = {h.num: h for h in self.sems.allocated().values()}
            for w in waits[1:]:
                extra = self.nc.sync.drain()
                extra.wait_op(handles[w.id], w.wait_value, "sem-ge")
        self.nc.all_engine_barrier()
        assert self.sems is not None
        popped = self.nc._tile_sem_poison_stack.pop()
        assert popped is self._sem_poison
        self.nc.clear_and_free_semaphores(list(self.sems.allocated().values()))
        self.nc.all_engine_barrier()

    tile.TileContext._drain_and_barrier = _patched
    tile.TileContext._drain_patched = True


def _split_excess_waits(nc, max_waits=1):
    """This walrus build encodes at most one sync-wait per instruction.
    Move extra waits onto same-engine NoOps inserted just before."""
    import bass_rust
    from concourse import mybir

    for f in nc.m.functions:
        for bb in f.blocks:
            new = []
            for ins in bb.instructions:
                si = ins.sync_info
                if si is not None and len(si.on_wait) > max_waits:
                    waits = list(si.on_wait)
                    for w in waits[: len(waits) - max_waits]:
                        nop = mybir.InstNoOp(
                            name=nc.get_next_instruction_name(), ins=[], outs=[]
                        )
                        nop.engine = ins.engine
                        h = bass_rust.SemaphoreHandle(name=w.ant_name, num=w.id)
                        bass_rust.wait_op(nop, h, w.wait_value, "sem-ge", False)
                        nc.register_instruction(nop, overwrite=True)
                        new.append(nop)
                    si.on_wait = waits[len(waits) - max_waits :]
                new.append(ins)
            bb.instructions = new


CFG = {
    "nchunks": 6,
    "ew16": True,
    "sq_engine": "act",
    "evict_engine": "act",
    "shift_engine": "dma",
    "gblock": 1,
    "grows": 9,
    "v16": False,
    "swdgeq": 4,
    "wdma": "sync",
    "sbufs": 2,
    "pbufs": 8,
    "cbufs": 2,
}


def _chunks_for(nchunks):
    """Partition input rows 0..55 into nchunks contiguous chunks such that
    matmul group r (needs input rows 9r..9r+10) only depends on chunks
    emitted at or before group r. Returns list of (r0, r1, first_group)
    where first_group is the earliest group index that must wait for it."""
    gper = [len(x) for x in np.array_split(np.arange(NGRP), nchunks)]
    out = []
    g0 = 0
    r_prev = 0
    for ng in gper:
        glast = g0 + ng - 1
        r1 = min(9 * glast + 11, H)
        out.append((r_prev, r1, g0))
        r_prev = r1
        g0 += ng
    return out


def _build(cfg=None):
    key = ("nc", tuple(sorted((cfg or CFG).items())))
    if key in _CACHE:
        return _CACHE[key]
    cfg = dict(CFG, **(cfg or {}))
    _patch_tile_tail_drain()
    import concourse.bass as bass
    import concourse.tile as tile
    from concourse import mybir

    f32 = mybir.dt.float32
    bf16 = mybir.dt.bfloat16
    ew = bf16 if cfg["ew16"] else f32
    Alu = mybir.AluOpType
    Act = mybir.ActivationFunctionType

    nc = bass.Bass("TRN2", num_swdge_queues=cfg["swdgeq"])
    x_d = nc.dram_tensor("x", [CIN, H, W], f32, kind="ExternalInput").ap()
    wspl_d = nc.dram_tensor(
        "wspl", [128, NTAP * NKT, 128], bf16, kind="ExternalInput"
    ).ap()
    # base weights arranged in concurrent row-tile pairs: pair j holds tap 2j
    # on partitions 0:64 and tap 2j+1 on 64:128 (tap 8 alone in pair 4).
    wbase_d = nc.dram_tensor("wbase2", [128, 5, 128], bf16, kind="ExternalInput").ap()
    bneg_d = nc.dram_tensor("betaneg", [128, NKT], f32, kind="ExternalInput").ap()
    y_d = nc.dram_tensor("y", [128, HO, WO], f32, kind="ExternalOutput").ap()

    WB = W * NKT  # 224: 4 k-tile column blocks side by side
    CB = cfg["cbufs"]

    nrep = cfg.get("replicate", 1)
    with tile.TileContext(nc) as tc:
        with (
            tc.tile_pool(name="consts", bufs=1) as cpool,
            tc.tile_pool(name="scratch", bufs=cfg["sbufs"]) as spool,
            tc.tile_pool(name="psum", bufs=cfg["pbufs"], space="PSUM") as ppool,
        ):
          for _rep in range(nrep):
              bneg = cpool.tile([128, NKT], f32, tag="bneg", bufs=CB)
              nc.sync.dma_start(bneg[:], bneg_d)
              # x first, row-chunked so chunk-0 elementwise starts immediately
              x2 = cpool.tile([128, H, W], f32, tag="x2", bufs=CB)
              nc.sync.dma_start(x2[0:CIN, 0:RR, :], x_d[:, 0:RR, :])
              nc.sync.dma_start(x2[CIN:128, 0:RR, :], x_d[:, 0:RR, :])
              wdma = nc.gpsimd.dma_start if cfg["wdma"] == "gpsimd" else nc.sync.dma_start
              wbase = cpool.tile([128, 5, 128], bf16, tag="wbase", bufs=CB)
              wdma(wbase[:], wbase_d)
              wspl = cpool.tile([128, NTAP * NKT, 128], bf16, tag="wspl", bufs=CB)
              wdma(wspl[:, 0:NTAP, :], wspl_d[:, 0:NTAP, :])
              nc.sync.dma_start(x2[0:CIN, RR:H, :], x_d[:, RR:H, :])
              nc.sync.dma_start(x2[CIN:128, RR:H, :], x_d[:, RR:H, :])
              wdma(wspl[:, NTAP : NKT * NTAP, :], wspl_d[:, NTAP : NKT * NTAP, :])

              silu2 = cpool.tile([128, H, W], bf16, tag="silu2", bufs=CB)
              siluB = cpool.tile([128, H, W], bf16, tag="siluB", bufs=CB)
              rhsW = cpool.tile([128, H, WB], bf16, tag="rhsW", bufs=CB)

              def emit_silu(r0, r1):
                  # lower = silu(x) on ACT; upper half is silu shifted (0,+1).
                  # x2's 64:128 partitions duplicate x, so the shifted half can
                  # be computed in-lane on ACT instead of a SBUF-SBUF DMA hop.
                  nc.scalar.activation(
                      silu2[0:CIN, r0:r1, :], x2[0:CIN, r0:r1, :], Act.Silu
                  )
                  if cfg["shift_engine"] == "act":
                      nc.scalar.activation(
                          silu2[CIN:128, r0:r1, 0 : W - 1],
                          x2[CIN:128, r0:r1, 1:W],
                          Act.Silu,
                      )
                  else:
                      nc.sync.dma_start(
                          silu2[CIN:128, r0:r1, 0 : W - 1],
                          silu2[0:CIN, r0:r1, 1:W],
                      )

              def emit_siluB(grp):
                  # S_B rows 9g..9g+8: lower = silu(x); upper shifted (+1,-2)
                  q0, q1 = 9 * grp, 9 * grp + GROW
                  if cfg["shift_engine"] == "act":
                      nc.scalar.activation(
                          siluB[CIN:128, q0:q1, 2:W],
                          x2[CIN:128, q0 + 1 : q1 + 1, 0 : W - 2],
                          Act.Silu,
                      )
                  else:
                      nc.sync.dma_start(
                          siluB[CIN:128, q0:q1, 2:W],
                          silu2[0:CIN, q0 + 1 : q1 + 1, 0 : W - 2],
                      )
                  nc.sync.dma_start(
                      siluB[0:CIN, q0:q1, :], silu2[0:CIN, q0:q1, :]
                  )

              def emit_chunk(r0, r1):
                  rows = r1 - r0
                  sl = (slice(None), slice(r0, r1), slice(0, WB))
                  v = spool.tile(
                      [128, rows, WB], ew if cfg["v16"] else f32, tag="v"
                  )
                  for t in range(NKT):
                      nc.scalar.activation(
                          v[:, :, t * W : (t + 1) * W],
                          x2[:, r0:r1, :],
                          Act.Abs,
                          bias=bneg[:, t : t + 1],
                          scale=2.5,
                      )
                  m = spool.tile([128, rows, WB], ew, tag="m")
                  nc.vector.tensor_scalar(m[:], v[:], 2.0, 2.0, Alu.min, Alu.subtract)
                  n = spool.tile([128, rows, WB], ew, tag="n")
                  nc.vector.tensor_scalar(n[:], v[:], 1.0, 1.0, Alu.min, Alu.subtract)
                  m2 = spool.tile([128, rows, WB], ew, tag="m2")
                  n2q = spool.tile([128, rows, WB], ew, tag="n2q")
                  if cfg["sq_engine"] == "act":
                      nc.scalar.activation(m2[:], m[:], Act.Square)
                      nc.scalar.activation(n2q[:], n[:], Act.Square, scale=2.0)
                  elif cfg["sq_engine"] == "dve":
                      nc.vector.tensor_tensor(m2[:], m[:], m[:], Alu.mult)
                      nc.vector.scalar_tensor_tensor(
                          n2q[:], n[:], 4.0, n[:], Alu.mult, Alu.mult
                      )
                  elif cfg["sq_engine"] == "pool":
                      nc.gpsimd.tensor_tensor(m2[:], m[:], m[:], Alu.mult)
                      nc.gpsimd.scalar_tensor_tensor(
                          n2q[:], n[:], 4.0, n[:], Alu.mult, Alu.mult
                      )
                  else:  # split: m2 on act, n2q on dve
                      nc.scalar.activation(m2[:], m[:], Act.Square)
                      nc.vector.scalar_tensor_tensor(
                          n2q[:], n[:], 4.0, n[:], Alu.mult, Alu.mult
                      )
                  m3 = spool.tile([128, rows, WB], ew, tag="m3")
                  nc.vector.tensor_tensor(m3[:], m2[:], m[:], Alu.mult)
                  n3q = spool.tile([128, rows, WB], ew, tag="n3q")
                  nc.vector.tensor_tensor(n3q[:], n2q[:], n[:], Alu.mult)
                  nc.vector.tensor_tensor(rhsW[sl], n3q[:], m3[:], Alu.subtract)

              chunks = _chunks_for(cfg["nchunks"])

              # MM blocking: `grows` output rows per matmul (9 or 18; free
              # dim grows*54 <= 1024 for bf16, PSUM tile spans 1-2 banks)
              GR = cfg["grows"]
              NFREE_B = GR * WO
              nblk = HO // GR

              def base_rv(blk, j):
                  # rhs window for base pair j (taps 2j/2j+1) of block blk
                  if j == 1:
                      return siluB[:, GR * blk : GR * blk + GR, 2 : 2 + WO]
                  if j == 4:
                      return silu2[
                          0:CIN, GR * blk + 2 : GR * blk + 2 + GR, 2 : 2 + WO
                      ]
                  kh, kw = divmod(2 * j, K)
                  return silu2[
                      :, GR * blk + kh : GR * blk + kh + GR, kw : kw + WO
                  ]

              def spline_rv(blk, t, tap):
                  kh, kw = divmod(tap, K)
                  return rhsW[
                      :,
                      GR * blk + kh : GR * blk + kh + GR,
                      t * W + kw : t * W + kw + WO,
                  ]

              def evict(blk, ps):
                  ev = spool.tile([128, NFREE_B], f32, tag="ev")
                  if cfg["evict_engine"] == "act":
                      nc.scalar.copy(ev[:], ps[:])
                  else:
                      nc.vector.tensor_copy(ev[:], ps[:])
                  nc.sync.dma_start(
                      y_d[:, GR * blk : GR * (blk + 1), :], ev[:]
                  )

              # weight tiles in issue order: (kind, idx) pairs
              wtiles = [("b", j) for j in (0, 2, 3, 4, 1)] + [
                  ("s", (t, tap)) for t in range(NKT) for tap in range(NTAP)
              ]
              gpb = NGRP // nblk  # 9-row groups per MM block
              GB = cfg["gblock"]
              for b0 in range(0, nblk, GB):
                  blks = range(b0, min(b0 + GB, nblk))
                  for blk in blks:
                      for grp in range(gpb * blk, gpb * (blk + 1)):
                          for (r0, r1, g0) in chunks:
                              if g0 == grp:
                                  emit_silu(r0, r1)
                                  emit_chunk(r0, r1)
                          emit_siluB(grp)
                  pss = {
                      blk: ppool.tile(
                          [128, NFREE_B], f32, tag="ps", name=f"ps{blk}"
                      )
                      for blk in blks
                  }
                  # weight-stationary over the block: one LDWEIGHTS feeds
                  # GB matmuls (one per block)
                  for wi, (kind, idx) in enumerate(wtiles):
                      if kind == "b":
                          lhsT = (
                              wbase[0:CIN, idx, :] if idx == 4
                              else wbase[:, idx, :]
                          )
                      else:
                          t, tap = idx
                          lhsT = wspl[:, t * NTAP + tap, :]
                      for blk in blks:
                          rv = (
                              base_rv(blk, idx) if kind == "b"
                              else spline_rv(blk, *idx)
                          )
                          nc.tensor.matmul(
                              pss[blk][:],
                              lhsT,
                              rv,
                              start=(wi == 0),
                              stop=(wi == len(wtiles) - 1),
                          )
                  for blk in blks:
                      evict(blk, pss[blk])

    _split_excess_waits(nc)
    _CACHE[key] = nc
    return nc


def _prep_weights(base_weight, spline_weight, spline_scaler):
    """Fold scaler and 1/6 into spline weights; lay out matmul lhsT tiles."""
    sw = (spline_weight * spline_scaler[:, :, None]).astype(np.float32) / 6.0
    # sw: [COUT, 576, 8]; feature index i = c*9 + tap
    sw4 = sw.reshape(COUT, CIN, NTAP, 8)  # [co, c, tap, g]
    # wspl[p, tap*4+t, co] = sw4[co, c, tap, 2t+gh], p = gh*64 + c
    w = np.transpose(sw4, (1, 2, 3, 0))  # [c, tap, g, co]
    w = w.reshape(CIN, NTAP, NKT, 2, COUT)  # g = 2t + gh -> [c, tap, t, gh, co]
    w = np.transpose(w, (3, 2, 0, 1, 4))  # [gh, t, c, tap, co]
    w = w.reshape(2, NKT, CIN, NTAP, COUT)
    w = np.transpose(w, (0, 2, 1, 3, 4))  # [gh, c, t, tap, co]
    wspl = w.reshape(2 * CIN, NKT * NTAP, COUT).astype(ml_dtypes.bfloat16)

    wb = base_weight.reshape(COUT, CIN, NTAP)  # [co, c, tap]
    wb_ct = np.transpose(wb, (1, 2, 0))  # [c, tap, co]
    wbase = np.zeros((128, 5, COUT), np.float32)
    for j in range(5):
        wbase[0:CIN, j, :] = wb_ct[:, 2 * j, :]
        if j < 4:
            wbase[CIN:128, j, :] = wb_ct[:, 2 * j + 1, :]
    wbase = wbase.astype(ml_dtypes.bfloat16)

    gh = np.arange(128) // CIN  # 0 for p<64, 1 otherwise
    t = np.arange(NKT)
    bneg = (3.5 - (2 * t[None, :] + gh[:, None])).astype(np.float32)  # [128, 4]
    return wspl, wbase, bneg


def _in_maps(x, base_weight, spline_weight, spline_scaler):
    wspl, wbase, bneg = _prep_weights(base_weight, spline_weight, spline_scaler)
    return [
        {
            "x": np.ascontiguousarray(x[b]).astype(np.float32),
            "wspl": wspl,
            "wbase2": wbase,
            "betaneg": bneg,
        }
        for b in range(B)
    ]


def kernel(x, base_weight, spline_weight, spline_scaler):
    from concourse.bass_utils import run_bass_kernel_spmd

    nc = _build()
    in_maps = _in_maps(x, base_weight, spline_weight, spline_scaler)
    res = run_bass_kernel_spmd(nc, in_maps, core_ids=list(range(B)))
    out = np.stack([res.results[b]["y"] for b in range(B)])  # [8, 128, 54, 54]
    return out.astype(np.float32)



# revision 9
# speedup vs baseline: 1.0163x; 1.0163x over previous
"""Trainium2 Bass kernel for CKANConv2d (KAN conv: SiLU base + B-spline path).

Math: for each output pixel p and output channel co:
  out[co,p] = sum_{c,kh,kw} silu(x[c,p+k]) * Wb[co,(c,kh,kw)]
            + sum_{c,kh,kw,g} B_g(x[c,p+k]) * Ws[co,(c,kh,kw),g]
with B_g the order-3 uniform B-spline bases over knots {-2.2 + 0.4j}.

On-chip closed form (t = 2.5*x, center c_g = g - 3.5, v = |t - c_g|):
  6*B_g = relu(2-v)^3 - 4*relu(1-v)^3
The 1/6 is folded into the spline weights; the bases are computed per
*input* pixel (not per unfolded patch, 9x less work) and the 3x3
convolution is done as an implicit GEMM over 9 shifted windows with
contraction (c,g) packed 128 rows at a time.

Elementwise chain (per 4-ktile block, all fp16 on DVE at 4x/2x rates):
  v   = |xs + bneg|          TS(add, abs_max), xs = 2.5*x from ACT
  t1  = 2 - v                TS(mult -1, add 2)
  t2  = cbrt(4)*(1 - v)      TS(mult -c4, add c4)
  q3  = relu(t1)^3           TS(max 0, pow 3)
  p3q = relu(t2)^3 = 4p^3    TS(max 0, pow 3)
  rhs = q3 - p3q             TT(subtract)
ACT only does xs, silu windows and PSUM eviction; the PE (bf16/fp16
1 cyc/row) is the busiest engine, ~50us/core of matmul columns.

Pipelining: every per-body tile rotates between 2 buffers (tag bufs=2),
so in the replicated timing build body i+1's input DMA + elementwise
basis computation overlap body i's tail matmuls and the PE never
starves. Base matmuls are emitted per-group so each group's PE block
depends only on that group's chunk of elementwise output.

Sharding: data-parallel over batch, 1 image per NeuronCore (8 cores).
"""
import numpy as np
import ml_dtypes

B, CIN, H, W = 8, 64, 56, 56
COUT, K = 128, 3
HO = WO = 54
NOUT = HO * WO  # 2916
NTAP = K * K  # 9
NKT = 4  # spline K-tiles per tap: 128 rows = 64c x 2g, 4 tiles cover g=0..7
NGRP = 6  # output row groups of 9 rows each
GROW = 9  # output rows per group
NFREE = GROW * WO  # 486 <= 512 (one PSUM bank)
C4 = float(4.0 ** (1.0 / 3.0))

_CACHE = {}


def _patch_tile_tail_drain():
    """walrus in this env rejects the Tile tail Drain when it carries >1
    sync waits; split them into a chain of single-wait Drains."""
    import concourse.tile as tile
    from concourse.vector_clock import ScopedClock

    if getattr(tile.TileContext, "_drain_patched", False):
        return

    def _patched(self, tick_clock, wait_clock):
        drain_inst = self.nc.sync.drain()
        wait_clock.add_sem_waits(
            drain_inst.ins, ScopedClock({None: tick_clock.global_clock})
        )
        si = drain_inst.ins.sync_info
        waits = list(si.on_wait) if si is not None else []
        if len(waits) > 1:
            si.on_wait = waits[:1]
            handles = {h.num: h for h in self.sems.allocated().values()}
            for w in waits[1:]:
                extra = self.nc.sync.drain()
                extra.wait_op(handles[w.id], w.wait_value, "sem-ge")
        self.nc.all_engine_barrier()
        assert self.sems is not None
        popped = self.nc._tile_sem_poison_stack.pop()
        assert popped is self._sem_poison
        self.nc.clear_and_free_semaphores(list(self.sems.allocated().values()))
        self.nc.all_engine_barrier()

    tile.TileContext._drain_and_barrier = _patched
    tile.TileContext._drain_patched = True


def _split_excess_waits(nc, max_waits=1):
    """This walrus build encodes at most one sync-wait per instruction.
    Move extra waits onto same-engine NoOps inserted just before."""
    import bass_rust
    from concourse import mybir

    for f in nc.m.functions:
        for bb in f.blocks:
            new = []
            for ins in bb.instructions:
                si = ins.sync_info
                if si is not None and len(si.on_wait) > max_waits:
                    waits = list(si.on_wait)
                    for w in waits[: len(waits) - max_waits]:
                        nop = mybir.InstNoOp(
                            name=nc.get_next_instruction_name(), ins=[], outs=[]
                        )
                        nop.engine = ins.engine
                        h = bass_rust.SemaphoreHandle(name=w.ant_name, num=w.id)
                        bass_rust.wait_op(nop, h, w.wait_value, "sem-ge", False)
                        nc.register_instruction(nop, overwrite=True)
                        new.append(nop)
                    si.on_wait = waits[len(waits) - max_waits :]
                new.append(ins)
            bb.instructions = new


KAN_OP_SHAS = {"v3": "096f51c5beb79f1f", "v4": "262e4c6f3fa8fc26"}


def _register_kan_op():
    """Custom fused DVE op: out = relu(s0 - in0)^3 - relu(in1)^3.
    With in0 = v = |2.5x - c_g|, in1 = cbrt(4)*(1 - v), s0 = 2.0 this is
    6*B_g(x) in one 8-stage DVE pass (the documented dve_ops extension
    point: append a DveOp to OPS)."""
    import concourse.dve_ops as dve_ops
    from concourse.dve_spec import Spec, Src0, Src1, C0, relu, sq

    for op in dve_ops.OPS:
        if op.name == "KAN_B3_COMBINE_ANT":
            return op

    def ref(in0, in1, s0, s1, imm2):
        q = np.maximum(s0 - in0.astype(np.float32), 0)
        p = np.maximum(in1.astype(np.float32), 0)
        return q * q * q - p * p * p

    q = relu(C0 - Src0)
    p = relu(Src1)
    op = dve_ops.DveOp(
        "KAN_B3_COMBINE_ANT",
        Spec(body=sq(q) * q - sq(p) * p, reference=ref),
        subdim=False,
        uops_sha=KAN_OP_SHAS,
    )
    dve_ops.OPS.append(op)
    dve_ops._SUB_OPCODE_FOR_NAME[op.name] = (
        dve_ops._CUSTOM_DVE_ROW_BASE + len(dve_ops.OPS) - 1
    )
    assert dve_ops._SUB_OPCODE_FOR_NAME[op.name] < 0x20
    return op


CFG = {
    "nchunks": 6,
    "chain": "cust",  # "cust": fused custom-DVE op; "sq": squares on ACT
    "ew": "f16",  # elementwise + matmul-rhs dtype: "f16" | "bf16"
    "shift_engine": "act",
    "evict_engine": "act",
    "gblock": 1,
    "grows": 9,
    "swdgeq": 4,
    "wdma": "sync",
    "sbufs": 2,
    "pbufs": 8,
    "cbufs": 2,
}


def _chunks_for(nchunks):
    """Partition input rows 0..55 into nchunks contiguous chunks such that
    matmul group r (needs input rows 9r..9r+10) only depends on chunks
    emitted at or before group r. Returns list of (r0, r1, first_group)
    where first_group is the earliest group index that must wait for it."""
    gper = [len(x) for x in np.array_split(np.arange(NGRP), nchunks)]
    out = []
    g0 = 0
    r_prev = 0
    for ng in gper:
        glast = g0 + ng - 1
        r1 = min(9 * glast + 11, H)
        out.append((r_prev, r1, g0))
        r_prev = r1
        g0 += ng
    return out


def _build(cfg=None):
    key = ("nc", tuple(sorted((cfg or CFG).items())))
    if key in _CACHE:
        return _CACHE[key]
    cfg = dict(CFG, **(cfg or {}))
    _patch_tile_tail_drain()
    kan_op = _register_kan_op()
    import concourse.bass as bass
    import concourse.tile as tile
    from concourse import mybir

    f32 = mybir.dt.float32
    ew = mybir.dt.float16 if cfg["ew"] == "f16" else mybir.dt.bfloat16
    Alu = mybir.AluOpType
    Act = mybir.ActivationFunctionType

    nc = bass.Bass("TRN2", num_swdge_queues=cfg["swdgeq"])
    x_d = nc.dram_tensor("x", [CIN, H, W], f32, kind="ExternalInput").ap()
    wspl_d = nc.dram_tensor(
        "wspl", [128, NTAP * NKT, 128], ew, kind="ExternalInput"
    ).ap()
    # base weights arranged in concurrent row-tile pairs: pair j holds tap 2j
    # on partitions 0:64 and tap 2j+1 on 64:128 (tap 8 alone in pair 4).
    wbase_d = nc.dram_tensor("wbase2", [128, 5, 128], ew, kind="ExternalInput").ap()
    bneg_d = nc.dram_tensor("betaneg", [128, NKT], f32, kind="ExternalInput").ap()
    y_d = nc.dram_tensor("y", [128, HO, WO], f32, kind="ExternalOutput").ap()

    WB = W * NKT  # 224: 4 k-tile column blocks side by side
    CB = cfg["cbufs"]
    RR = 12  # first x-DMA rows (chunk 0 of nchunks=6; fine for any nchunks)

    nrep = cfg.get("replicate", 1)
    with tile.TileContext(nc) as tc:
        with (
            tc.tile_pool(name="consts", bufs=1) as cpool,
            tc.tile_pool(name="scratch", bufs=cfg["sbufs"]) as spool,
            tc.tile_pool(name="psum", bufs=cfg["pbufs"], space="PSUM") as ppool,
        ):
          for _rep in range(nrep):
              bneg = cpool.tile([128, NKT], f32, tag="bneg", bufs=CB)
              nc.sync.dma_start(bneg[:], bneg_d)
              # x first, row-chunked so chunk-0 elementwise starts immediately
              x2 = cpool.tile([128, H, W], f32, tag="x2", bufs=CB)
              nc.sync.dma_start(x2[0:CIN, 0:RR, :], x_d[:, 0:RR, :])
              nc.sync.dma_start(x2[CIN:128, 0:RR, :], x_d[:, 0:RR, :])
              wdma = nc.gpsimd.dma_start if cfg["wdma"] == "gpsimd" else nc.sync.dma_start
              wbase = cpool.tile([128, 5, 128], ew, tag="wbase", bufs=CB)
              wdma(wbase[:], wbase_d)
              wspl = cpool.tile([128, NTAP * NKT, 128], ew, tag="wspl", bufs=CB)
              wdma(wspl[:, 0:NTAP, :], wspl_d[:, 0:NTAP, :])
              nc.sync.dma_start(x2[0:CIN, RR:H, :], x_d[:, RR:H, :])
              nc.sync.dma_start(x2[CIN:128, RR:H, :], x_d[:, RR:H, :])
              wdma(wspl[:, NTAP : NKT * NTAP, :], wspl_d[:, NTAP : NKT * NTAP, :])

              silu2 = cpool.tile([128, H, W], ew, tag="silu2", bufs=CB)
              siluB = cpool.tile([128, H, W], ew, tag="siluB", bufs=CB)
              rhsW = cpool.tile([128, H, WB], ew, tag="rhsW", bufs=CB)

              def emit_silu(r0, r1):
                  # lower = silu(x) on ACT; upper half is silu shifted (0,+1).
                  # x2's 64:128 partitions duplicate x, so the shifted half can
                  # be computed in-lane on ACT instead of a SBUF-SBUF DMA hop.
                  nc.scalar.activation(
                      silu2[0:CIN, r0:r1, :], x2[0:CIN, r0:r1, :], Act.Silu
                  )
                  if cfg["shift_engine"] == "act":
                      nc.scalar.activation(
                          silu2[CIN:128, r0:r1, 0 : W - 1],
                          x2[CIN:128, r0:r1, 1:W],
                          Act.Silu,
                      )
                  else:
                      nc.sync.dma_start(
                          silu2[CIN:128, r0:r1, 0 : W - 1],
                          silu2[0:CIN, r0:r1, 1:W],
                      )

              def emit_siluB(grp):
                  # S_B rows 9g..9g+8: lower = silu(x); upper = silu shifted
                  # (+1,-2). Both straight from x2 on ACT (no silu2 dep).
                  q0, q1 = 9 * grp, 9 * grp + GROW
                  if cfg["shift_engine"] == "act":
                      nc.scalar.activation(
                          siluB[CIN:128, q0:q1, 2:W],
                          x2[CIN:128, q0 + 1 : q1 + 1, 0 : W - 2],
                          Act.Silu,
                      )
                      nc.scalar.activation(
                          siluB[0:CIN, q0:q1, :], x2[0:CIN, q0:q1, :], Act.Silu
                      )
                  else:
                      nc.sync.dma_start(
                          siluB[CIN:128, q0:q1, 2:W],
                          silu2[0:CIN, q0 + 1 : q1 + 1, 0 : W - 2],
                      )
                      nc.sync.dma_start(
                          siluB[0:CIN, q0:q1, :], silu2[0:CIN, q0:q1, :]
                      )

              def emit_chunk(r0, r1):
                  rows = r1 - r0
                  sl = (slice(None), slice(r0, r1), slice(0, WB))
                  v = spool.tile([128, rows, WB], ew, tag="v")
                  for t in range(NKT):
                      nc.scalar.activation(
                          v[:, :, t * W : (t + 1) * W],
                          x2[:, r0:r1, :],
                          Act.Abs,
                          bias=bneg[:, t : t + 1],
                          scale=2.5,
                      )
                  if cfg["chain"] == "cust":
                      t2 = spool.tile([128, rows, WB], ew, tag="t2")
                      nc.vector.tensor_scalar(
                          t2[:], v[:], -C4, C4, Alu.mult, Alu.add
                      )
                      nc.vector._custom_dve(
                          kan_op, out=rhsW[sl], in0=v[:], in1=t2[:], s0=2.0
                      )
                  else:  # "sq": squares on ACT, cubes+combine on DVE
                      m = spool.tile([128, rows, WB], ew, tag="m")
                      nc.vector.tensor_scalar(
                          m[:], v[:], 2.0, 2.0, Alu.min, Alu.subtract
                      )
                      n = spool.tile([128, rows, WB], ew, tag="n")
                      nc.vector.tensor_scalar(
                          n[:], v[:], 1.0, 1.0, Alu.min, Alu.subtract
                      )
                      m2 = spool.tile([128, rows, WB], ew, tag="m2")
                      nc.scalar.activation(m2[:], m[:], Act.Square)
                      n2q = spool.tile([128, rows, WB], ew, tag="n2q")
                      nc.scalar.activation(n2q[:], n[:], Act.Square, scale=2.0)
                      m3 = spool.tile([128, rows, WB], ew, tag="m3")
                      nc.vector.tensor_tensor(m3[:], m2[:], m[:], Alu.mult)
                      n3q = spool.tile([128, rows, WB], ew, tag="n3q")
                      nc.vector.tensor_tensor(n3q[:], n2q[:], n[:], Alu.mult)
                      nc.vector.tensor_tensor(
                          rhsW[sl], n3q[:], m3[:], Alu.subtract
                      )

              chunks = _chunks_for(cfg["nchunks"])

              # MM blocking: `grows` output rows per matmul (9 or 18; free
              # dim grows*54 <= 1024 for bf16, PSUM tile spans 1-2 banks)
              GR = cfg["grows"]
              NFREE_B = GR * WO
              nblk = HO // GR

              def base_rv(blk, j):
                  # rhs window for base pair j (taps 2j/2j+1) of block blk
                  if j == 1:
                      return siluB[:, GR * blk : GR * blk + GR, 2 : 2 + WO]
                  if j == 4:
                      return silu2[
                          0:CIN, GR * blk + 2 : GR * blk + 2 + GR, 2 : 2 + WO
                      ]
                  kh, kw = divmod(2 * j, K)
                  return silu2[
                      :, GR * blk + kh : GR * blk + kh + GR, kw : kw + WO
                  ]

              def spline_rv(blk, t, tap):
                  kh, kw = divmod(tap, K)
                  return rhsW[
                      :,
                      GR * blk + kh : GR * blk + kh + GR,
                      t * W + kw : t * W + kw + WO,
                  ]

              def evict(blk, ps):
                  ev = spool.tile([128, NFREE_B], f32, tag="ev")
                  if cfg["evict_engine"] == "act":
                      nc.scalar.copy(ev[:], ps[:])
                  else:
                      nc.vector.tensor_copy(ev[:], ps[:])
                  nc.sync.dma_start(
                      y_d[:, GR * blk : GR * (blk + 1), :], ev[:]
                  )

              # weight tiles in issue order: (kind, idx) pairs
              wtiles = [("b", j) for j in (0, 2, 3, 4, 1)] + [
                  ("s", (t, tap)) for t in range(NKT) for tap in range(NTAP)
              ]
              gpb = NGRP // nblk  # 9-row groups per MM block
              GB = cfg["gblock"]
              for b0 in range(0, nblk, GB):
                  blks = range(b0, min(b0 + GB, nblk))
                  for blk in blks:
                      for grp in range(gpb * blk, gpb * (blk + 1)):
                          for (r0, r1, g0) in chunks:
                              if g0 == grp:
                                  emit_silu(r0, r1)
                                  emit_chunk(r0, r1)
                          emit_siluB(grp)
                  pss = {
                      blk: ppool.tile(
                          [128, NFREE_B], f32, tag="ps", name=f"ps{blk}"
                      )
                      for blk in blks
                  }
                  # weight-stationary over the block: one LDWEIGHTS feeds
                  # GB matmuls (one per block)
                  for wi, (kind, idx) in enumerate(wtiles):
                      if kind == "b":
                          lhsT = (
                              wbase[0:CIN, idx, :] if idx == 4
                              else wbase[:, idx, :]
                          )
                      else:
                          t, tap = idx
                          lhsT = wspl[:, t * NTAP + tap, :]
                      for blk in blks:
                          rv = (
                              base_rv(blk, idx) if kind == "b"
                              else spline_rv(blk, *idx)
                          )
                          nc.tensor.matmul(
                              pss[blk][:],
                              lhsT,
                              rv,
                              start=(wi == 0),
                              stop=(wi == len(wtiles) - 1),
                          )
                  for blk in blks:
                      evict(blk, pss[blk])

    _split_excess_waits(nc)
    # populate .instr bytes for InstCustomDveAnt (raw Bass skips the Bacc
    # codegen pass that does this; without it walrus sees "ISA wrong length")
    mybir.codegen_inst_isa_subclasses(nc)
    _CACHE[key] = nc
    return nc


def _prep_weights(base_weight, spline_weight, spline_scaler, ewnp):
    """Fold scaler and 1/6 into spline weights; lay out matmul lhsT tiles."""
    sw = (spline_weight * spline_scaler[:, :, None]).astype(np.float32) / 6.0
    # sw: [COUT, 576, 8]; feature index i = c*9 + tap
    sw4 = sw.reshape(COUT, CIN, NTAP, 8)  # [co, c, tap, g]
    # wspl[p, tap*4+t, co] = sw4[co, c, tap, 2t+gh], p = gh*64 + c
    w = np.transpose(sw4, (1, 2, 3, 0))  # [c, tap, g, co]
    w = w.reshape(CIN, NTAP, NKT, 2, COUT)  # g = 2t + gh -> [c, tap, t, gh, co]
    w = np.transpose(w, (3, 2, 0, 1, 4))  # [gh, t, c, tap, co]
    w = w.reshape(2, NKT, CIN, NTAP, COUT)
    w = np.transpose(w, (0, 2, 1, 3, 4))  # [gh, c, t, tap, co]
    wspl = w.reshape(2 * CIN, NKT * NTAP, COUT).astype(ewnp)

    wb = base_weight.reshape(COUT, CIN, NTAP)  # [co, c, tap]
    wb_ct = np.transpose(wb, (1, 2, 0))  # [c, tap, co]
    wbase = np.zeros((128, 5, COUT), np.float32)
    for j in range(5):
        wbase[0:CIN, j, :] = wb_ct[:, 2 * j, :]
        if j < 4:
            wbase[CIN:128, j, :] = wb_ct[:, 2 * j + 1, :]
    wbase = wbase.astype(ewnp)

    gh = np.arange(128) // CIN  # 0 for p<64, 1 otherwise
    t = np.arange(NKT)
    bneg = (3.5 - (2 * t[None, :] + gh[:, None])).astype(np.float32)  # [128, 4]
    return wspl, wbase, bneg


def _in_maps(x, base_weight, spline_weight, spline_scaler):
    ewnp = ml_dtypes.bfloat16 if CFG["ew"] == "bf16" else np.float16
    wspl, wbase, bneg = _prep_weights(base_weight, spline_weight, spline_scaler, ewnp)
    return [
        {
            "x": np.ascontiguousarray(x[b]).astype(np.float32),
            "wspl": wspl,
            "wbase2": wbase,
            "betaneg": bneg,
        }
        for b in range(B)
    ]


def kernel(x, base_weight, spline_weight, spline_scaler):
    from concourse.bass_utils import run_bass_kernel_spmd

    nc = _build()
    in_maps = _in_maps(x, base_weight, spline_weight, spline_scaler)
    res = run_bass_kernel_spmd(nc, in_maps, core_ids=list(range(B)))
    out = np.stack([res.results[b]["y"] for b in range(B)])  # [8, 128, 54, 54]
    return out.astype(np.float32)
